# revision 38
# baseline (speedup 1.0000x reference)
"""GCN layer kernel for 8 Trainium2 NeuronCores.

out = segment_sum(edge_vals * (features @ W)[edge_src], edge_dst) + bias
    = segment_sum(edge_vals * features[edge_src], edge_dst) @ W + bias
      (W is shared across nodes, so aggregation commutes with the matmul)

Strategy (graph/data parallel per sharding hint):
- Destination nodes are sharded across 8 cores (12500 per core), in 98
  windows of 128 dsts (window = psum accumulation target).
- Edges of a window are packed densely into columns of 128 (partition =
  edge lane, NOT dst). Batched SWDGE dma_gather fetches many columns per
  instruction (int16 indices relative to a per-instruction bank base;
  4 banks of 32768 nodes cover the table; idx=0/val=0 pads). Gathers
  round-robin over 4 SWDGE queues so their DMA transfers overlap.
- Features are pre-converted to bf16 on the host: halves gather bytes
  and runs the PE at full bf16 rate.
- Per gather instruction, two broadcast tensor_tensor ops build
  Sel[e, c, d] = val[e,c] * (d == dst[e,c]) for all chunk columns at
  once (2-byte dtypes -> 2x DVE). Per column, one PE matmul accumulates
  agg^T[f,d] += sum_e G[e,f] * Sel[e,d] into the window's PSUM bank --
  scale, scatter and reduce in one op.
- Per window: Scalar engine copies agg^T to SBUF (bf16), PE matmul with
  W stationary produces (agg @ W)^T, Scalar engine adds bias
  (per-partition now that fout is the partition dim), DMA out. The
  transposed windows are fixed up on the host.
- Edges sorted by source within each bank run for HBM locality.
"""
import os
import sys
from contextlib import ExitStack

import numpy as np

_REPO = "/opt/trn_rl_repo"
if _REPO not in sys.path:
    sys.path.insert(0, _REPO)

N_NODES = 100000
N_EDGES = 3200000
DIM = 128
N_CORES = 8
P = 128
SHARD = N_NODES // N_CORES  # 12500
N_WIN = (SHARD + P - 1) // P  # 98 windows/core
SHARD_PAD = N_WIN * P  # 12544
BANK = 32768
N_BANKS = (N_NODES + BANK - 1) // BANK  # 4
# PSUM accumulation tiles are bank-granular: 6 window banks + 2 pso banks.
# k-major column interleaving within a (group, bank) run still makes
# consecutive columns sweep the same source range (DRAM row reuse).
WGRP = int(os.environ.get("GCN_WGRP", "6"))  # windows per gather group
# 8 cols = 1024 idxs = 65 descs per DMA engine; 2 fit in the 128-entry
# SWDGE ring. 16 cols would be 129 descs/engine and overflows the ring
# (NRT_EXEC_UNIT_UNRECOVERABLE).
CHUNK = int(os.environ.get("GCN_CHUNK", "8"))  # columns per dma_gather inst
SCRATCH = int(os.environ.get("GCN_SCRATCH", "16384"))
NQ = int(os.environ.get("GCN_NQ", "4"))  # SWDGE queues


def _host_schedule(edge_src, edge_dst, edge_vals):
    """Pack each core's edges into dense 128-lane columns.

    Column order: for each group g of WGRP windows, for each bank b, for
    each window w in g: that window's bank-b edges (sorted by src) in
    chunks of 128, last column padded with idx=0/val=0. Column counts are
    forced identical across cores (shared program) by padding to the max
    per (g, b, w) run.
    """
    import ml_dtypes

    core = edge_dst // SHARD
    per_core = []
    for c in range(N_CORES):
        m = core == c
        src_c = np.asarray(edge_src[m], dtype=np.int64)
        dst_c = np.asarray(edge_dst[m] - c * SHARD, dtype=np.int64)
        val_c = np.asarray(edge_vals[m], dtype=np.float32)
        w = dst_c // P
        b = src_c // BANK
        g = w // WGRP
        order = np.lexsort((src_c, w, b, g))
        per_core.append(
            {"src": src_c[order], "dst": dst_c[order], "val": val_c[order],
             "w": w[order], "b": b[order], "g": g[order]}
        )
    n_grp = (N_WIN + WGRP - 1) // WGRP
    counts = np.zeros((N_CORES, n_grp, N_BANKS, N_WIN), dtype=np.int64)
    for c in range(N_CORES):
        pc = per_core[c]
        np.add.at(counts[c], (pc["g"], pc["b"], pc["w"]), 1)
    ncols_run = (np.max(counts, axis=0) + P - 1) // P  # [n_grp, N_BANKS, N_WIN]
    for w in range(N_WIN):
        g = w // WGRP
        if ncols_run[g, :, w].sum() == 0:  # window must still write bias
            ncols_run[g, 0, w] = 1

    cols = []     # (w, b) per column
    insts = []    # (bank, col_start, n_cols)
    col_pos = {}  # (g, b, w) -> global column indices of that run (k-major)
    for g in range(n_grp):
        wlist = list(range(g * WGRP, min((g + 1) * WGRP, N_WIN)))
        for b in range(N_BANKS):
            # Interleave windows k-major: column k of every window covers the
            # same source-rank slice (edges are src-sorted per run), so
            # consecutive gather fetches revisit the same DRAM rows.
            run_cols = []
            kmax = max((int(ncols_run[g, b, w]) for w in wlist), default=0)
            for k in range(kmax):
                for w in wlist:
                    if k < ncols_run[g, b, w]:
                        col_pos.setdefault((g, b, w), []).append(
                            len(cols) + len(run_cols)
                        )
                        run_cols.append((w, b))
            i = 0
            while i < len(run_cols):
                nc_i = min(CHUNK, len(run_cols) - i)
                insts.append((b, len(cols) + i, nc_i))
                i += nc_i
            cols += run_cols
    total_cols = len(cols)

    for c in range(N_CORES):
        pc = per_core[c]
        idx_cells = np.zeros((total_cols, P), dtype=np.int16)
        dstid = np.zeros((P, total_cols), dtype=np.float32)
        val = np.zeros((P, total_cols), dtype=np.float32)
        colreal = np.zeros(total_cols, dtype=np.int64)  # real lanes per column
        key = pc["g"] * (N_BANKS * N_WIN) + pc["b"] * N_WIN + pc["w"]
        uniq, starts_idx = np.unique(key, return_index=True)
        boundaries = np.concatenate([starts_idx, [len(key)]])
        for k, u in enumerate(uniq):
            lo, hi = boundaries[k], boundaries[k + 1]
            g = int(u) // (N_BANKS * N_WIN)
            b = (int(u) // N_WIN) % N_BANKS
            w = int(u) % N_WIN
            n = hi - lo
            src_rel = (pc["src"][lo:hi] - b * BANK).astype(np.int16)
            d_loc = (pc["dst"][lo:hi] - w * P).astype(np.float32)
            v = pc["val"][lo:hi]
            ncol = (n + P - 1) // P
            pad = ncol * P - n
            flat_idx = np.concatenate([src_rel, np.zeros(pad, np.int16)])
            flat_d = np.concatenate([d_loc, np.zeros(pad, np.float32)])
            flat_v = np.concatenate([v, np.zeros(pad, np.float32)])
            poss = np.array(col_pos[(g, b, w)][:ncol])
            idx_cells[poss] = flat_idx.reshape(ncol, P)
            dstid[:, poss] = flat_d.reshape(ncol, P).T
            val[:, poss] = flat_v.reshape(ncol, P).T
            colreal[poss[:-1]] = P
            colreal[poss[-1]] = n - (ncol - 1) * P
        # wrapped int16 stream per instruction: [128, total_cols*8]
        idx16 = np.zeros((128, total_cols * 8), dtype=np.int16)
        for (b, c0, nc_i) in insts:
            L = idx_cells[c0:c0 + nc_i].reshape(-1)
            wrapped = np.zeros((16, nc_i * 8), dtype=np.int16)
            ii = np.arange(nc_i * P)
            wrapped[ii % 16, ii // 16] = L
            idx16[:, c0 * 8:(c0 + nc_i) * 8] = np.tile(wrapped, (8, 1))
        if os.environ.get("GCN_ZIDX", "0") == "1":  # perf probe: all row-0
            idx16[:] = 0
        if os.environ.get("GCN_NARROW", "0") == "1":  # perf probe: hot rows
            idx16[:] = idx16 % 2048
        # NOTE: trimming pad descriptors via num_idxs_reg-from-register (with
        # trailing idx=-1) matches the interp contract but crashes the HW
        # ucode (NRT_EXEC_UNIT_UNRECOVERABLE); only static num_idxs is safe.
        pc["idx16"] = idx16
        pc["dstid"] = dstid.astype(ml_dtypes.bfloat16)
        pc["val"] = val.astype(ml_dtypes.bfloat16)
    return cols, insts, total_cols, per_core


def _build_nc(cols, insts, total_cols):
    import concourse.bass as bass
    import concourse.tile as tile
    from concourse import bacc, mybir, library_config

    nc = bacc.Bacc(
        "TRN2", target_bir_lowering=False, debug=False, num_devices=N_CORES,
        dynamic_dma_scratch_size=SCRATCH, num_swdge_queues=NQ,
    )
    feat_t = nc.dram_tensor("features16", [N_NODES, DIM], mybir.dt.bfloat16, kind="ExternalInput")
    idx_t = nc.dram_tensor("idx16", [P, total_cols * 8], mybir.dt.int16, kind="ExternalInput")
    dst_t = nc.dram_tensor("dstid", [P, total_cols], mybir.dt.bfloat16, kind="ExternalInput")
    val_t = nc.dram_tensor("val", [P, total_cols], mybir.dt.bfloat16, kind="ExternalInput")
    w_t = nc.dram_tensor("weight16", [DIM, DIM], mybir.dt.bfloat16, kind="ExternalInput")
    bias_t = nc.dram_tensor("bias_col", [P, 1], mybir.dt.float32, kind="ExternalInput")
    iota_t = nc.dram_tensor("iota", [P, P], mybir.dt.bfloat16, kind="ExternalInput")
    out_t = nc.dram_tensor("outp", [N_WIN, DIM, P], mybir.dt.float32, kind="ExternalOutput")

    first_col = {}
    last_col = {}
    for ci, (w, b) in enumerate(cols):
        if w not in first_col:
            first_col[w] = ci
        last_col[w] = ci

    with tile.TileContext(nc) as tc:
        with ExitStack() as ctx:
            nc.gpsimd.load_library(library_config.mlp)
            const = ctx.enter_context(tc.tile_pool(name="const", bufs=1))
            ipool = ctx.enter_context(tc.tile_pool(name="idx", bufs=16))
            gpool = ctx.enter_context(tc.tile_pool(name="gather", bufs=12))
            spool = ctx.enter_context(tc.tile_pool(name="sel", bufs=12))
            opool = ctx.enter_context(tc.tile_pool(name="outw", bufs=6))
            tppool = ctx.enter_context(tc.tile_pool(name="aggt", bufs=8))
            # PSUM slots are bank-granular: 6 window banks + 2 pso banks.
            # (7 window banks + 1 pso bank measured SLOWER: the single pso
            # bank serializes the per-window drain chains.)
            pswin = ctx.enter_context(
                tc.tile_pool(name="psw", bufs=WGRP, space="PSUM")
            )
            psopool = ctx.enter_context(
                tc.tile_pool(name="pso", bufs=2, space="PSUM")
            )

            dst_all = const.tile([P, total_cols], mybir.dt.bfloat16)
            nc.sync.dma_start(dst_all[:], dst_t[:])
            val_all = const.tile([P, total_cols], mybir.dt.bfloat16)
            nc.sync.dma_start(val_all[:], val_t[:])
            w_tile = const.tile([DIM, DIM], mybir.dt.bfloat16)
            nc.sync.dma_start(w_tile[:], w_t[:])
            bias_tile = const.tile([P, 1], mybir.dt.float32)
            nc.sync.dma_start(bias_tile[:], bias_t[:])
            iota = const.tile([P, P], mybir.dt.bfloat16)
            nc.sync.dma_start(iota[:], iota_t[:])

            # Zero the gather pool slots once so any lane a truncated or
            # padded gather leaves stale never multiplies as a NaN pattern.
            for _ in range(12):
                gz = gpool.tile([P, CHUNK * P], mybir.dt.bfloat16, name="G")
                nc.vector.memset(gz[:], 0.0)

            ps_of_win = {}
            for qi, (b, c0, nc_i) in enumerate(insts):
                idxs = ipool.tile([P, nc_i * 8], mybir.dt.int16, name="idxs")
                nc.sync.dma_start(idxs[:], idx_t[:, c0 * 8:(c0 + nc_i) * 8])
                G = gpool.tile([P, nc_i * P], mybir.dt.bfloat16, name="G")
                g3 = G[:].rearrange("p (c f) -> p c f", f=P)
                nc.gpsimd.dma_gather(
                    out_ap=g3,
                    in_ap=feat_t[b * BANK:][:],
                    idxs_ap=idxs[:],
                    num_idxs=nc_i * P,
                    num_idxs_reg=nc_i * P,
                    elem_size=DIM,
                    queue_num=qi % NQ,
                )
                # Sel[e, c, d] = val[e, c] * (iota[d] == dst[e, c]) for the
                # whole chunk in two broadcast DVE ops (all 2-byte -> 2x).
                sel = spool.tile([P, nc_i, P], mybir.dt.bfloat16, name="sel")
                iota_bc = iota[:].unsqueeze(1).broadcast_to([P, nc_i, P])
                dst_bc = (
                    dst_all[:, c0:c0 + nc_i].unsqueeze(2).broadcast_to([P, nc_i, P])
                )
                val_bc = (
                    val_all[:, c0:c0 + nc_i].unsqueeze(2).broadcast_to([P, nc_i, P])
                )
                nc.vector.tensor_tensor(
                    out=sel[:], in0=iota_bc, in1=dst_bc,
                    op=mybir.AluOpType.is_equal,
                )
                nc.vector.tensor_tensor(
                    out=sel[:], in0=sel[:], in1=val_bc,
                    op=mybir.AluOpType.mult,
                )
                for j in range(nc_i):
                    ci = c0 + j
                    w = cols[ci][0]
                    if w not in ps_of_win:
                        ps_of_win[w] = pswin.tile(
                            [P, P], mybir.dt.float32, name="pst"
                        )
                    # agg^T[f, d] += sum_e G[e, f] * Sel[e, d]
                    nc.tensor.matmul(
                        out=ps_of_win[w][:],
                        lhsT=G[:, j * P:(j + 1) * P],
                        rhs=sel[:, j, :],
                        start=(ci == first_col[w]),
                        stop=(ci == last_col[w]),
                    )
                    if ci == last_col[w]:
                        agg_tr = tppool.tile([P, P], mybir.dt.bfloat16)
                        nc.scalar.activation(
                            agg_tr[:], ps_of_win[w][:],
                            mybir.ActivationFunctionType.Copy,
                        )
                        ps_o = psopool.tile([P, DIM], mybir.dt.float32, name="pso")
                        # (agg @ W)^T[fout, d] = sum_f W[f, fout] * agg^T[f, d]
                        nc.tensor.matmul(
                            out=ps_o[:], lhsT=w_tile[:], rhs=agg_tr[:],
                            start=True, stop=True,
                        )
                        ow = opool.tile([P, DIM], mybir.dt.float32)
                        nc.scalar.activation(
                            ow[:], ps_o[:],
                            mybir.ActivationFunctionType.Identity,
                            bias=bias_tile[:],
                        )
                        nc.sync.dma_start(out_t[w], ow[:])
                        del ps_of_win[w]
    nc.compile()
    return nc


def kernel(features, edge_src, edge_dst, edge_vals, weight, bias):
    import ml_dtypes

    features = np.ascontiguousarray(np.asarray(features), dtype=np.float32)
    edge_src = np.asarray(edge_src).astype(np.int64)
    edge_dst = np.asarray(edge_dst).astype(np.int64)
    edge_vals = np.asarray(edge_vals).astype(np.float32)
    weight = np.asarray(weight).astype(np.float32)
    bias = np.asarray(bias).astype(np.float32)

    cols, insts, total_cols, per_core = _host_schedule(edge_src, edge_dst, edge_vals)
    nc = _build_nc(cols, insts, total_cols)

    from concourse.bass_utils import run_bass_kernel_spmd

    feat16 = features.astype(ml_dtypes.bfloat16)
    w16 = weight.astype(ml_dtypes.bfloat16)
    bias_col = np.ascontiguousarray(bias[:, None]).astype(np.float32)
    iota = np.tile(
        np.arange(P, dtype=np.float32)[None, :], (P, 1)
    ).astype(ml_dtypes.bfloat16)
    in_maps = []
    for c in range(N_CORES):
        in_maps.append(
            {
                "features16": feat16,
                "idx16": per_core[c]["idx16"],
                "dstid": per_core[c]["dstid"],
                "val": per_core[c]["val"],
                "weight16": w16,
                "bias_col": bias_col,
                "iota": iota,
            }
        )
    trace = os.environ.get("GCN_TRACE", "0") == "1"
    res = None
    for attempt in range(3):
        try:
            res = run_bass_kernel_spmd(
                nc, in_maps, core_ids=list(range(N_CORES)), trace=trace
            )
            break
        except Exception:
            if attempt == 2:
                raise
            import time as _time

            _time.sleep(15.0)  # transient device flakes recover across retries
    if trace:
        print(f"HW exec time: {res.exec_time_ns} ns")
        kernel.last_exec_time_ns = res.exec_time_ns

    out = np.empty((N_NODES, DIM), dtype=np.float32)
    for c in range(N_CORES):
        op = res.results[c]["outp"]  # [N_WIN, DIM, P] transposed windows
        op = np.ascontiguousarray(op.transpose(0, 2, 1)).reshape(SHARD_PAD, DIM)
        out[c * SHARD:(c + 1) * SHARD] = op[:SHARD]
    return out


kernel.last_exec_time_ns = None
